# revision 6
# baseline (speedup 1.0000x reference)
"""BitLinear (int8-activation x ternary-weight) matmul on 8 TRN2 NeuronCores.

Full inputs: x [4, 4096, 2048] f32, weight [2048, 2048] f32.
Output: [4, 4096, 2048] fp16 = ((qx @ qw.T) / si / sw).astype(f16).

Strategy: data-parallel over the 16384 rows (2048 rows/core); W is
replicated. The mean|W| pass runs as ACT Abs (f32->bf16) + TensorE
ones-matmul column-sum accumulation, so the 16 MB W scan is purely
DMA-bound, the DVE stays free for activation prequant, and the PE's
HAM clock gate is held open by real work. NCACHE W k-tiles stay
resident in SBUF; the rest are re-read as half-tiles on the vector
DMA queue, issued early enough to stream in behind the cached-tile
quantization. W quant is one DVE magic-round (w*sw + 1.5*2^23) plus
one ACT Sign(u - MAGIC) straight to fp8 (sign(n) == clip(n,-1,1) for
integer n). Per-row activation quantization to int8 values held in
bf16 uses the same magic trick and a DMA-xbar block transpose; four
row tiles are prequantized during the W scan. The matmul runs
bf16(lhsT=qx^T) x fp8(qw^T) with fp32 PSUM accumulation (exact) and
the dequant (acc * amax/127 * mean|W|) fused into the PSUM->SBUF fp16
copy. The first two row tiles interleave across k so each quantized W
k-tile unlocks 8 matmuls during the ramp; a 16-matmul burst keyed on
the last abs tile re-warms the clock right before the stream. Host
only reshapes/shards and transposes W (layout prep, no math).
"""

import numpy as np

import concourse.mybir as mybir
import concourse.tile as tile
from concourse import bacc
from concourse.bass import ts
from concourse.bass_utils import run_bass_kernel_spmd

N_CORES = 8
ROWS_TOTAL = 4 * 4096
K = 2048
N = 2048
NCACHE = 7  # W k-tiles kept in SBUF between the mean pass and quantize pass
MAGIC = 12582912.0  # 1.5*2^23: fp32 round-to-nearest-even (both signs)

f32 = mybir.dt.float32
bf16 = mybir.dt.bfloat16
f16 = mybir.dt.float16
fp8 = mybir.dt.float8e4
Alu = mybir.AluOpType
Act = mybir.ActivationFunctionType
AxX = mybir.AxisListType.X


def build(rows_per_core=ROWS_TOTAL // N_CORES):
    nc = bacc.Bacc(
        "TRN2", target_bir_lowering=False, debug=False, num_devices=N_CORES
    )
    x_ext = nc.declare_dram_parameter("x", [rows_per_core, K], f32, isOutput=False)
    wt_ext = nc.declare_dram_parameter("wt", [K, N], f32, isOutput=False)
    out_ext = nc.declare_dram_parameter(
        "out", [rows_per_core, N], f16, isOutput=True
    )

    KT = K // 128
    MT = rows_per_core // 128
    NQ = N // 512
    NPRE = min(4, MT)  # x tiles prefetched + prequantized during the W scan

    with tile.TileContext(nc) as tc:
        with (
            tc.tile_pool(name="xin", bufs=2) as xin,  # [128,K] f32 x loads
            tc.tile_pool(name="wfst", bufs=2) as wfst,  # non-cached W 1st read
            tc.tile_pool(name="wch", bufs=NCACHE) as wch,  # cached W tiles
            tc.tile_pool(name="wre", bufs=3) as wre,  # [128,1024] re-read halves
            tc.tile_pool(name="wabs", bufs=2) as wabs,  # [128,K] bf16 |W|
            tc.tile_pool(name="scaled", bufs=2) as scaled,  # [128,K] f32 ACT out
            tc.tile_pool(name="qtmp", bufs=2) as qtmp,  # qx bf16
            tc.tile_pool(name="qxt", bufs=4) as qxtp,  # [128,KT,128] bf16 x^T
            tc.tile_pool(name="outp", bufs=2) as outp,  # [128,N] f16 results
            tc.tile_pool(name="singles", bufs=1) as singles,
            tc.tile_pool(name="small", bufs=6) as small,  # [128,1] stats
            tc.tile_pool(name="pacc", bufs=8, space="PSUM") as pacc,
        ):
            ones_mat = singles.tile([128, 128], bf16)
            nc.vector.memset(ones_mat, 1.0)
            negmagic_b = singles.tile([128, 1], f32)
            nc.vector.memset(negmagic_b, -MAGIC)
            qwT = singles.tile([128, KT, N], fp8)

            # ---- W scan: DMA each k-tile, ACT |.|->bf16, accumulate the
            # column sums on the PE (ones^T @ |w|). Every partition of
            # psum_mean ends up holding the same per-column totals, so the
            # final X-reduce yields the grand total replicated [128,1]
            # with no extra broadcast. x tiles for the prequant ride the
            # same queue, interleaved so W still finishes first-ish.
            psum_mean = pacc.tile([128, 512], f32, tag="acc", name="pmean")
            x_pre = {}
            w_tiles = {}
            abs_tiles = {}

            def issue_x(mi):
                x_t = xin.tile([128, K], f32, tag="xin", name=f"x{mi}")
                nc.sync.dma_start(out=x_t, in_=x_ext[ts(mi, 128), :])
                x_pre[mi] = x_t

            x_slots = {4: 0, 7: 1, 9: 2, 11: 3}  # after this kt, issue x_i
            mm_i = 0
            for kt in range(KT):
                if kt < NCACHE:
                    wt_t = wch.tile([128, K], f32, tag="wch", name=f"wch{kt}")
                else:
                    wt_t = wfst.tile([128, K], f32, tag="wfst", name=f"wf{kt}")
                nc.sync.dma_start(out=wt_t, in_=wt_ext[ts(kt, 128), :])
                w_tiles[kt] = wt_t
                if kt in x_slots and x_slots[kt] < NPRE:
                    issue_x(x_slots[kt])
                a_t = wabs.tile([128, K], bf16, tag="wabs", name=f"wa{kt}")
                nc.scalar.activation(out=a_t, in_=wt_t, func=Act.Abs)
                abs_tiles[kt] = a_t
                for sl in range(4):
                    nc.tensor.matmul(
                        psum_mean,
                        lhsT=ones_mat,
                        rhs=a_t[:, ts(sl, 512)],
                        start=(mm_i == 0),
                        stop=(mm_i == 4 * KT - 1),
                    )
                    mm_i += 1

            # re-read DMAs for the non-cached k-tiles, as half-tiles on the
            # otherwise-idle gpsimd DMA queue: the 3-buf WAR stall then
            # blocks nothing else, and the first halves pre-stage during
            # the scan so the quant pass never waits on a cold queue
            wre_tiles = {}
            for kt in range(NCACHE, KT):
                for h in range(2):
                    r_t = wre.tile([128, 1024], f32, tag="wre", name=f"wr{kt}_{h}")
                    nc.gpsimd.dma_start(
                        out=r_t, in_=wt_ext[ts(kt, 128), ts(h, 1024)]
                    )
                    wre_tiles[(kt, h)] = r_t

            def x_quant(mi):
                if mi in x_pre:
                    x_t = x_pre.pop(mi)
                else:
                    x_t = xin.tile([128, K], f32, tag="xin", name=f"x{mi}")
                    nc.sync.dma_start(out=x_t, in_=x_ext[ts(mi, 128), :])
                amax = small.tile([128, 1], f32, tag="small")
                nc.vector.tensor_reduce(
                    out=amax, in_=x_t, axis=AxX, op=Alu.max,
                    apply_absolute_value=True,
                )
                amc = small.tile([128, 1], f32, tag="amc", name=f"amc{mi}")
                nc.vector.tensor_scalar_max(out=amc, in0=amax, scalar1=1e-5)
                rec = small.tile([128, 1], f32, tag="small")
                nc.vector.reciprocal(out=rec, in_=amc)
                si = small.tile([128, 1], f32, tag="small")
                nc.vector.tensor_scalar_mul(out=si, in0=rec, scalar1=127.0)
                xs = scaled.tile([128, K], f32, tag="scaled")
                nc.scalar.activation(out=xs, in_=x_t, func=Act.Copy, scale=si)
                qx = qtmp.tile([128, K], bf16, tag="qtmp")
                nc.vector.tensor_scalar(
                    out=qx, in0=xs, scalar1=MAGIC, scalar2=-MAGIC,
                    op0=Alu.add, op1=Alu.add,
                )
                qxT = qxtp.tile(
                    [128, KT, 128], bf16, tag="qxt", name=f"qxT{mi}"
                )
                nc.sync.dma_start_transpose(out=qxT, in_=qx)
                return qxT, amc

            # prequantize the first NPRE row tiles while the W scan runs
            xq = [x_quant(mi) for mi in range(NPRE)]

            # ---- sw from the PE-accumulated |W| sums
            tot = small.tile([128, 1], f32, tag="s1")
            nc.vector.tensor_reduce(out=tot, in_=psum_mean, axis=AxX, op=Alu.add)
            meanc_b = small.tile([128, 1], f32, tag="s1b")
            nc.vector.tensor_scalar(
                out=meanc_b, in0=tot, scalar1=1.0 / (K * N), scalar2=1e-5,
                op0=Alu.mult, op1=Alu.max,
            )
            sw_b = singles.tile([128, 1], f32)
            nc.vector.reciprocal(out=sw_b, in_=meanc_b)
            q_b = singles.tile([128, 1], f32)
            nc.vector.tensor_scalar_mul(out=q_b, in0=meanc_b, scalar1=1.0 / 127.0)

            # ---- HAM re-warm burst: 16 junk matmuls keyed on the last abs
            # tile, so they run right before the ramp (the mean matmuls
            # during the scan are only ~35% duty cycle)
            pwarms = [
                pacc.tile([128, 512], f32, tag="acc", name=f"warm{i}")
                for i in range(2)
            ]
            for wi in range(12):
                nc.tensor.matmul(
                    pwarms[wi % 2], lhsT=ones_mat,
                    rhs=abs_tiles[KT - 1][:, :512],
                    start=True, stop=True, skip_group_check=True,
                )

            # ---- W pass 2: qwT = sign(round(wT*sw)) as fp8, cached tiles
            # first so the re-read halves have time to arrive
            def quant_w(src, kt, n0, width):
                nc.vector.tensor_scalar(
                    out=src, in0=src, scalar1=sw_b, scalar2=MAGIC,
                    op0=Alu.mult, op1=Alu.add,
                )
                nc.scalar.activation(
                    out=qwT[:, kt, n0 : n0 + width], in_=src,
                    func=Act.Sign, bias=negmagic_b,
                )

            for kt in range(NCACHE):
                quant_w(w_tiles[kt], kt, 0, K)
            for kt in range(NCACHE, KT):
                for h in range(2):
                    quant_w(wre_tiles[(kt, h)], kt, h * 1024, 1024)

            # ---- main loop over row tiles
            def mm(acc, qxT, kt, nq):
                nc.tensor.matmul(
                    acc, lhsT=qxT[:, kt, :], rhs=qwT[:, kt, ts(nq, 512)],
                    start=(kt == 0), stop=(kt == KT - 1),
                    skip_group_check=True,
                )

            def finish(mi, accs, amc):
                cs = small.tile([128, 1], f32, tag="small")
                nc.vector.tensor_mul(cs, amc, q_b)  # (amax/127)*meanc
                o_t = outp.tile([128, N], f16, tag="outp", name=f"o{mi}")
                for nq in range(NQ):
                    nc.scalar.activation(
                        out=o_t[:, ts(nq, 512)], in_=accs[nq],
                        func=Act.Copy, scale=cs,
                    )
                nc.scalar.dma_start(out=out_ext[ts(mi, 128), :], in_=o_t)

            if MT >= 2:
                # interleave the first two row tiles across kt so each
                # quantized qwT k-tile unlocks 8 matmuls during the ramp
                qxT0, amc0 = xq[0]
                qxT1, amc1 = xq[1]
                accs0 = [
                    pacc.tile([128, 512], f32, tag="acc", name=f"acc_0_{i}")
                    for i in range(NQ)
                ]
                accs1 = [
                    pacc.tile([128, 512], f32, tag="acc", name=f"acc_1_{i}")
                    for i in range(NQ)
                ]
                for kt in range(KT):
                    for nq in range(NQ):
                        mm(accs0[nq], qxT0, kt, nq)
                    for nq in range(NQ):
                        mm(accs1[nq], qxT1, kt, nq)
                finish(0, accs0, amc0)
                finish(1, accs1, amc1)
                start_mi = 2
            else:
                start_mi = 0

            for mi in range(start_mi, MT):
                qxT, amc = xq[mi] if mi < NPRE else x_quant(mi)
                accs = [
                    pacc.tile([128, 512], f32, tag="acc", name=f"acc_{mi}_{i}")
                    for i in range(NQ)
                ]
                if mi == MT - 1:
                    # nq-inner: each output chunk completes as soon as its
                    # 16 accumulations are done, so the dequant + store
                    # overlap the remaining matmuls (shorter kernel tail)
                    for nq in range(NQ):
                        for kt in range(KT):
                            mm(accs[nq], qxT, kt, nq)
                else:
                    for kt in range(KT):
                        for nq in range(NQ):
                            mm(accs[nq], qxT, kt, nq)
                finish(mi, accs, amc)

    nc.compile()
    return nc


_NC_CACHE = {}


def _get_nc(rows_per_core):
    if rows_per_core not in _NC_CACHE:
        _NC_CACHE[rows_per_core] = build(rows_per_core)
    return _NC_CACHE[rows_per_core]


def run(x, weight, **spmd_kwargs):
    x = np.ascontiguousarray(np.asarray(x, dtype=np.float32))
    weight = np.asarray(weight, dtype=np.float32)
    b, s, k = x.shape
    rows = b * s
    rpc = rows // N_CORES
    xr = x.reshape(rows, k)
    wt = np.ascontiguousarray(weight.T)
    nc = _get_nc(rpc)
    in_maps = [
        {"x": xr[i * rpc : (i + 1) * rpc], "wt": wt} for i in range(N_CORES)
    ]
    res = run_bass_kernel_spmd(
        nc, in_maps, core_ids=list(range(N_CORES)), **spmd_kwargs
    )
    out = np.concatenate(
        [res.results[i]["out"] for i in range(N_CORES)], axis=0
    )
    return out.reshape(b, s, N), res


def kernel(x, weight):
    out, _ = run(x, weight)
    return out
